# revision 16
# baseline (speedup 1.0000x reference)
"""Trainium2 Bass kernel for nn_Attention_41686952575337.

Computes, per batch b (B=8, one NeuronCore each, weights replicated):
    q = Wq @ x1[b] + bq            (K=128, Lq=2048)
    k = Wk @ x2[b] + bk            (K=128, Lk=2048)
    v = Wv @ x2[b] + bv            (O=128, Lk=2048)
    u = (k^T q) / sqrt(K)          (Lk, Lq)
    w = softmax(u, axis=0)         (softmax over Lk)
    out = v @ w                    (O, Lq)
returns (out, w) stacked over batch.

Strategy: scores live in (m=Lk on partitions, q on free) tiles. The
softmax-over-partitions sum runs on the TensorEngine with an all-ones
stationary operand (which also broadcasts the denominator across all 128
partitions); exp on ScalarE; normalize split between VectorE and GpSimd
(both are otherwise idle halves of the time). Big matmuls run as float32r
(full-rate fp32 path, ~1e-4 accuracy). m-tiles are processed in PAIRS
sharing one (128,1024) PSUM tile so ACT/DVE instructions are twice as
large (per-instruction overhead and semaphore traffic dominate otherwise).
Inputs are loaded in chunks with k projected first so the score stream
starts early; the last q-chunk is normalized at half-tile grain to shrink
the kernel tail.
"""

import numpy as np

B = 8
C = 128          # CQ = CM = K = O = 128
LQ = 2048
LK = 2048
P = 128
QCHUNK = 512     # q-chunk width (one PSUM bank)
NQ = LQ // QCHUNK
NM = LK // P     # 16 m-tiles
NPAIR = NM // 2  # m-tile pairs
SCALE = 1.0 / float(np.sqrt(np.float32(C)))

_CACHE = {}


def _build_program():
    import concourse.tile as tile
    from concourse import bacc, mybir
    from concourse.masks import make_identity

    f32 = mybir.dt.float32
    f32r = mybir.dt.float32r
    Exp = mybir.ActivationFunctionType.Exp
    Ident = mybir.ActivationFunctionType.Identity

    nc = bacc.Bacc(None, target_bir_lowering=False)

    x1 = nc.dram_tensor("x1", [C, LQ], f32, kind="ExternalInput")
    x2 = nc.dram_tensor("x2", [C, LK], f32, kind="ExternalInput")
    wq_d = nc.dram_tensor("Wq", [C, C], f32, kind="ExternalInput")
    bq_d = nc.dram_tensor("bq", [C], f32, kind="ExternalInput")
    wk_d = nc.dram_tensor("Wk", [C, C], f32, kind="ExternalInput")
    bk_d = nc.dram_tensor("bk", [C], f32, kind="ExternalInput")
    wv_d = nc.dram_tensor("Wv", [C, C], f32, kind="ExternalInput")
    bv_d = nc.dram_tensor("bv", [C], f32, kind="ExternalInput")
    out_d = nc.dram_tensor("out", [C, LQ], f32, kind="ExternalOutput")
    w_d = nc.dram_tensor("w", [LK, LQ], f32, kind="ExternalOutput")
    # view for paired stores: (m-pair, row, half, q)
    w_pairs = w_d.rearrange("(t h p) q -> t p h q", h=2, p=P)

    with tile.TileContext(nc) as tc:
        with (
            tc.tile_pool(name="singles", bufs=1) as singles,
            tc.tile_pool(name="big", bufs=1) as big,
            tc.tile_pool(name="exps", bufs=3) as exps,
            tc.tile_pool(name="post", bufs=2) as post,
            tc.tile_pool(name="wout", bufs=8) as wout,
            tc.tile_pool(name="pu", bufs=3, space="PSUM") as pu,
            tc.tile_pool(name="pacc", bufs=1, space="PSUM") as pacc,
        ):
            # ---- constants / weights ----
            ident = singles.tile([P, P], f32)
            make_identity(nc, ident)
            ones_f = singles.tile([P, P], f32)
            nc.vector.memset(ones_f, 1.0)
            ones = singles.tile([P, P], f32r)
            nc.vector.tensor_copy(ones, ones_f)

            x1_sb = big.tile([C, LQ], f32, tag="x1")
            x2_sb = big.tile([C, LK], f32, tag="x2")
            for j in range(2):
                sl = slice(j * LQ // 2, (j + 1) * LQ // 2)
                nc.sync.dma_start(x2_sb[:, sl], x2[:, sl])
            wgt = singles.tile([P, 3, C], f32)       # Wq | Wk | Wv
            nc.gpsimd.dma_start(wgt[:, 0, :], wq_d[:])
            nc.gpsimd.dma_start(wgt[:, 1, :], wk_d[:])
            nc.gpsimd.dma_start(wgt[:, 2, :], wv_d[:])
            bias = singles.tile([P, 3], f32)         # bq | bk | bv
            nc.gpsimd.dma_start(bias[:, 0:1], bq_d[:, None])
            nc.gpsimd.dma_start(bias[:, 1:2], bk_d[:, None])
            nc.gpsimd.dma_start(bias[:, 2:3], bv_d[:, None])
            bqs = singles.tile([P, 1], f32)          # bq / sqrt(K)
            nc.vector.tensor_scalar_mul(bqs, bias[:, 0:1], SCALE)

            # transpose weights (lhsT layout: contraction on partitions)
            wgtT = singles.tile([P, 3, C], f32)      # WqT | WkT | WvT
            ptw = pu.tile([P, 2 * QCHUNK], f32, tag="pu")
            for i in range(3):
                nc.tensor.transpose(ptw[:, i * P:(i + 1) * P], wgt[:, i, :], ident)
            nc.vector.tensor_copy(wgtT[:], ptw[:, :3 * P].rearrange("p (i c) -> p i c", i=3))

            # ---- chunked input loads; x2/k first so the score stream can
            # start as early as possible ----
            for j in range(2):
                sl = slice(j * LQ // 2, (j + 1) * LQ // 2)
                nc.sync.dma_start(x1_sb[:, sl], x1[:, sl])

            # ---- projections (exact fp32 matmuls; ACT rounds to f32r) ----
            q_sb = big.tile([P, LQ], f32r, tag="q")
            k_sb = big.tile([P, LK], f32r, tag="k")
            v_sb = big.tile([P, LK], f32, tag="v")
            for (dst, wi, b_ap, scl, src) in (
                (k_sb, 1, bias[:, 1:2], 1.0, x2_sb),
                (v_sb, 2, bias[:, 2:3], 1.0, x2_sb),
                (q_sb, 0, bqs, SCALE, x1_sb),
            ):
                for j in range(NQ // 2):
                    pp = pu.tile([P, 2 * QCHUNK], f32, tag="pu")
                    for h in range(2):
                        sl = slice((2 * j + h) * QCHUNK, (2 * j + h + 1) * QCHUNK)
                        nc.tensor.matmul(pp[:, h * QCHUNK:(h + 1) * QCHUNK],
                                         wgtT[:, wi, :], src[:, sl],
                                         start=True, stop=True)
                    dsl = slice(2 * j * QCHUNK, 2 * (j + 1) * QCHUNK)
                    nc.scalar.activation(dst[:, dsl], pp, Ident, bias=b_ap, scale=scl)

            # ---- v^T tiles: vT[:, mi*128:+128] = v[:, mi-chunk]^T ----
            vT_sb = big.tile([P, LK], f32r, tag="vT")
            for g in range(2):
                ptv = pu.tile([P, 2 * QCHUNK], f32, tag="pu")
                for i in range(8):
                    mi = g * 8 + i
                    nc.tensor.transpose(ptv[:, i * P:(i + 1) * P],
                                        v_sb[:, mi * P:(mi + 1) * P], ident)
                nc.vector.tensor_copy(vT_sb[:, g * 8 * P:(g + 1) * 8 * P], ptv)

            # ---- main loop over q-chunks ----
            for qc in range(NQ):
                qsl = slice(qc * QCHUNK, (qc + 1) * QCHUNK)
                po = pacc.tile([P, QCHUNK], f32, tag="po")    # out accum (unnorm)
                pd = pacc.tile([P, QCHUNK], f32, tag="pd")    # denominator (bcast)
                exp_tiles = []
                for t in range(NPAIR):
                    pu_t = pu.tile([P, 2 * QCHUNK], f32, tag="pu")
                    for h in range(2):
                        msl = slice((2 * t + h) * P, (2 * t + h + 1) * P)
                        nc.tensor.matmul(pu_t[:, h * QCHUNK:(h + 1) * QCHUNK],
                                         k_sb[:, msl], q_sb[:, qsl],
                                         start=True, stop=True)
                    e_t = exps.tile([P, 2 * QCHUNK], f32r, tag=f"exp_{t}")
                    nc.scalar.activation(e_t, pu_t, Exp)
                    exp_tiles.append(e_t)
                    for h in range(2):
                        hs = slice(h * QCHUNK, (h + 1) * QCHUNK)
                        msl = slice((2 * t + h) * P, (2 * t + h + 1) * P)
                        first = t == 0 and h == 0
                        last = t == NPAIR - 1 and h == 1
                        nc.tensor.matmul(pd, ones, e_t[:, hs],
                                         start=first, stop=last)
                        nc.tensor.matmul(po, vT_sb[:, msl], e_t[:, hs],
                                         start=first, stop=last)
                recip = post.tile([P, QCHUNK], f32, tag="recip")
                nc.vector.reciprocal_approx_fast(out=recip, in_=pd)
                # out-normalize first: frees the po/pd accumulators for the
                # next chunk before the long w-normalize stream
                o_sb = post.tile([P, QCHUNK], f32, tag="osb")
                nc.vector.tensor_mul(o_sb, po, recip)
                nc.sync.dma_start(out_d[:, qsl], o_sb)
                # normalize + store w (DVE; GpSimd shares SBUF ports so
                # offloading there halves both engines); finer grain on the
                # last chunk to shrink the kernel tail
                if qc < NQ - 1:
                    rbc = recip[:, None, :].to_broadcast([P, 2, QCHUNK])
                    for t in range(NPAIR):
                        w_t = wout.tile([P, 2, QCHUNK], f32, tag="w")
                        nc.vector.tensor_mul(
                            w_t,
                            exp_tiles[t].bitcast(f32).rearrange("p (h q) -> p h q", h=2),
                            rbc)
                        nc.sync.dma_start(w_pairs[t, :, :, qsl], w_t)
                else:
                    for t in range(NPAIR):
                        for h in range(2):
                            hs = slice(h * QCHUNK, (h + 1) * QCHUNK)
                            mi = 2 * t + h
                            w_t = wout.tile([P, QCHUNK], f32, tag="wh")
                            nc.vector.tensor_mul(
                                w_t, exp_tiles[t].bitcast(f32)[:, hs], recip)
                            nc.sync.dma_start(w_d[mi * P:(mi + 1) * P, qsl], w_t)

    nc.finalize()
    return nc


def _get_program():
    if "nc" not in _CACHE:
        _CACHE["nc"] = _build_program()
    return _CACHE["nc"]


def kernel(x1, x2, Wq, bq, Wk, bk, Wv, bv):
    from concourse.bass_utils import run_bass_kernel_spmd

    nc = _get_program()
    in_maps = []
    for b in range(B):
        in_maps.append({
            "x1": np.ascontiguousarray(x1[b], dtype=np.float32),
            "x2": np.ascontiguousarray(x2[b], dtype=np.float32),
            "Wq": np.ascontiguousarray(Wq, dtype=np.float32),
            "bq": np.ascontiguousarray(bq, dtype=np.float32),
            "Wk": np.ascontiguousarray(Wk, dtype=np.float32),
            "bk": np.ascontiguousarray(bk, dtype=np.float32),
            "Wv": np.ascontiguousarray(Wv, dtype=np.float32),
            "bv": np.ascontiguousarray(bv, dtype=np.float32),
        })
    res = run_bass_kernel_spmd(nc, in_maps, core_ids=list(range(B)),
                               trace=_CACHE.get("trace", False))
    _CACHE["last_result"] = res
    out = np.stack([res.results[b]["out"] for b in range(B)])
    w = np.stack([res.results[b]["w"] for b in range(B)])
    return out, w


# revision 17
# speedup vs baseline: 1.0386x; 1.0386x over previous
"""Trainium2 Bass kernel for nn_Attention_41686952575337.

Computes, per batch b (B=8, one NeuronCore each, weights replicated):
    q = Wq @ x1[b] + bq            (K=128, Lq=2048)
    k = Wk @ x2[b] + bk            (K=128, Lk=2048)
    v = Wv @ x2[b] + bv            (O=128, Lk=2048)
    u = (k^T q) / sqrt(K)          (Lk, Lq)
    w = softmax(u, axis=0)         (softmax over Lk)
    out = v @ w                    (O, Lq)
returns (out, w) stacked over batch.

Strategy: scores live in (m=Lk on partitions, q on free) tiles. The
softmax-over-partitions sum runs on the TensorEngine with an all-ones
stationary operand (which also broadcasts the denominator across all 128
partitions); exp on ScalarE; normalize split between VectorE and GpSimd
(both are otherwise idle halves of the time). Big matmuls run as float32r
(full-rate fp32 path, ~1e-4 accuracy). m-tiles are processed in PAIRS
sharing one (128,1024) PSUM tile so ACT/DVE instructions are twice as
large (per-instruction overhead and semaphore traffic dominate otherwise).
Inputs are loaded in chunks with k projected first so the score stream
starts early; the last q-chunk is normalized at half-tile grain to shrink
the kernel tail.
"""

import numpy as np

B = 8
C = 128          # CQ = CM = K = O = 128
LQ = 2048
LK = 2048
P = 128
QCHUNK = 512     # q-chunk width (one PSUM bank)
NQ = LQ // QCHUNK
NM = LK // P     # 16 m-tiles
NPAIR = NM // 2  # m-tile pairs
SCALE = 1.0 / float(np.sqrt(np.float32(C)))

_CACHE = {}


def _build_program():
    import concourse.tile as tile
    from concourse import bacc, mybir
    from concourse.masks import make_identity

    f32 = mybir.dt.float32
    f32r = mybir.dt.float32r
    Exp = mybir.ActivationFunctionType.Exp
    Ident = mybir.ActivationFunctionType.Identity

    nc = bacc.Bacc(None, target_bir_lowering=False)

    x1 = nc.dram_tensor("x1", [C, LQ], f32, kind="ExternalInput")
    x2 = nc.dram_tensor("x2", [C, LK], f32, kind="ExternalInput")
    wq_d = nc.dram_tensor("Wq", [C, C], f32, kind="ExternalInput")
    bq_d = nc.dram_tensor("bq", [C], f32, kind="ExternalInput")
    wk_d = nc.dram_tensor("Wk", [C, C], f32, kind="ExternalInput")
    bk_d = nc.dram_tensor("bk", [C], f32, kind="ExternalInput")
    wv_d = nc.dram_tensor("Wv", [C, C], f32, kind="ExternalInput")
    bv_d = nc.dram_tensor("bv", [C], f32, kind="ExternalInput")
    out_d = nc.dram_tensor("out", [C, LQ], f32, kind="ExternalOutput")
    w_d = nc.dram_tensor("w", [LK, LQ], f32, kind="ExternalOutput")
    # view for paired stores: (m-pair, row, half, q)
    w_pairs = w_d.rearrange("(t h p) q -> t p h q", h=2, p=P)

    with tile.TileContext(nc) as tc:
        with (
            tc.tile_pool(name="singles", bufs=1) as singles,
            tc.tile_pool(name="big", bufs=1) as big,
            tc.tile_pool(name="exps", bufs=3) as exps,
            tc.tile_pool(name="post", bufs=2) as post,
            tc.tile_pool(name="wout", bufs=8) as wout,
            tc.tile_pool(name="pu", bufs=3, space="PSUM") as pu,
            tc.tile_pool(name="pacc", bufs=1, space="PSUM") as pacc,
        ):
            # ---- constants / weights ----
            ident = singles.tile([P, P], f32)
            make_identity(nc, ident)
            ones_f = singles.tile([P, P], f32)
            nc.vector.memset(ones_f, 1.0)
            ones = singles.tile([P, P], f32r)
            nc.vector.tensor_copy(ones, ones_f)

            x1_sb = big.tile([C, LQ], f32, tag="x1")
            x2_sb = big.tile([C, LK], f32, tag="x2")
            for j in range(NQ):
                sl = slice(j * QCHUNK, (j + 1) * QCHUNK)
                nc.sync.dma_start(x2_sb[:, sl], x2[:, sl])
            wgt = singles.tile([P, 3, C], f32)       # Wq | Wk | Wv
            nc.sync.dma_start(wgt[:, 0, :], wq_d[:])
            nc.sync.dma_start(wgt[:, 1, :], wk_d[:])
            nc.sync.dma_start(wgt[:, 2, :], wv_d[:])
            bias = singles.tile([P, 3], f32)         # bq | bk | bv
            nc.sync.dma_start(bias[:, 0:1], bq_d[:, None])
            nc.sync.dma_start(bias[:, 1:2], bk_d[:, None])
            nc.sync.dma_start(bias[:, 2:3], bv_d[:, None])
            bqs = singles.tile([P, 1], f32)          # bq / sqrt(K)
            nc.vector.tensor_scalar_mul(bqs, bias[:, 0:1], SCALE)

            # transpose weights (lhsT layout: contraction on partitions)
            wgtT = singles.tile([P, 3, C], f32)      # WqT | WkT | WvT
            ptw = pu.tile([P, 2 * QCHUNK], f32, tag="pu")
            for i in range(3):
                nc.tensor.transpose(ptw[:, i * P:(i + 1) * P], wgt[:, i, :], ident)
            nc.vector.tensor_copy(wgtT[:], ptw[:, :3 * P].rearrange("p (i c) -> p i c", i=3))

            # ---- chunked input loads; x2/k first so the score stream can
            # start as early as possible ----
            for j in range(NQ):
                sl = slice(j * QCHUNK, (j + 1) * QCHUNK)
                nc.sync.dma_start(x1_sb[:, sl], x1[:, sl])

            # ---- projections (exact fp32 matmuls; ACT rounds to f32r) ----
            q_sb = big.tile([P, LQ], f32r, tag="q")
            k_sb = big.tile([P, LK], f32r, tag="k")
            v_sb = big.tile([P, LK], f32, tag="v")
            for (dst, wi, b_ap, scl, src) in (
                (k_sb, 1, bias[:, 1:2], 1.0, x2_sb),
                (v_sb, 2, bias[:, 2:3], 1.0, x2_sb),
                (q_sb, 0, bqs, SCALE, x1_sb),
            ):
                for j in range(NQ // 2):
                    pp = pu.tile([P, 2 * QCHUNK], f32, tag="pu")
                    for h in range(2):
                        sl = slice((2 * j + h) * QCHUNK, (2 * j + h + 1) * QCHUNK)
                        nc.tensor.matmul(pp[:, h * QCHUNK:(h + 1) * QCHUNK],
                                         wgtT[:, wi, :], src[:, sl],
                                         start=True, stop=True)
                    dsl = slice(2 * j * QCHUNK, 2 * (j + 1) * QCHUNK)
                    nc.scalar.activation(dst[:, dsl], pp, Ident, bias=b_ap, scale=scl)

            # ---- v^T tiles: vT[:, mi*128:+128] = v[:, mi-chunk]^T ----
            vT_sb = big.tile([P, LK], f32r, tag="vT")
            for g in range(2):
                ptv = pu.tile([P, 2 * QCHUNK], f32, tag="pu")
                for i in range(8):
                    mi = g * 8 + i
                    nc.tensor.transpose(ptv[:, i * P:(i + 1) * P],
                                        v_sb[:, mi * P:(mi + 1) * P], ident)
                nc.vector.tensor_copy(vT_sb[:, g * 8 * P:(g + 1) * 8 * P], ptv)

            # ---- main loop over q-chunks ----
            for qc in range(NQ):
                qsl = slice(qc * QCHUNK, (qc + 1) * QCHUNK)
                po = pacc.tile([P, QCHUNK], f32, tag="po")    # out accum (unnorm)
                pd = pacc.tile([P, QCHUNK], f32, tag="pd")    # denominator (bcast)
                exp_tiles = []
                for t in range(NPAIR):
                    pu_t = pu.tile([P, 2 * QCHUNK], f32, tag="pu")
                    for h in range(2):
                        msl = slice((2 * t + h) * P, (2 * t + h + 1) * P)
                        nc.tensor.matmul(pu_t[:, h * QCHUNK:(h + 1) * QCHUNK],
                                         k_sb[:, msl], q_sb[:, qsl],
                                         start=True, stop=True)
                    e_t = exps.tile([P, 2 * QCHUNK], f32r, tag=f"exp_{t}")
                    nc.scalar.activation(e_t, pu_t, Exp)
                    exp_tiles.append(e_t)
                    for h in range(2):
                        hs = slice(h * QCHUNK, (h + 1) * QCHUNK)
                        msl = slice((2 * t + h) * P, (2 * t + h + 1) * P)
                        first = t == 0 and h == 0
                        last = t == NPAIR - 1 and h == 1
                        nc.tensor.matmul(pd, ones, e_t[:, hs],
                                         start=first, stop=last)
                        nc.tensor.matmul(po, vT_sb[:, msl], e_t[:, hs],
                                         start=first, stop=last)
                recip = post.tile([P, QCHUNK], f32, tag="recip")
                nc.vector.reciprocal_approx_fast(out=recip, in_=pd)
                # out-normalize first: frees the po/pd accumulators for the
                # next chunk before the long w-normalize stream
                o_sb = post.tile([P, QCHUNK], f32, tag="osb")
                nc.vector.tensor_mul(o_sb, po, recip)
                nc.sync.dma_start(out_d[:, qsl], o_sb)
                # normalize + store w (DVE; GpSimd shares SBUF ports so
                # offloading there halves both engines); finer grain on the
                # last chunk to shrink the kernel tail
                if qc < NQ - 1:
                    rbc = recip[:, None, :].to_broadcast([P, 2, QCHUNK])
                    for t in range(NPAIR):
                        w_t = wout.tile([P, 2, QCHUNK], f32, tag="w")
                        nc.vector.tensor_mul(
                            w_t,
                            exp_tiles[t].bitcast(f32).rearrange("p (h q) -> p h q", h=2),
                            rbc)
                        nc.sync.dma_start(w_pairs[t, :, :, qsl], w_t)
                else:
                    for t in range(NPAIR):
                        for h in range(2):
                            hs = slice(h * QCHUNK, (h + 1) * QCHUNK)
                            mi = 2 * t + h
                            w_t = wout.tile([P, QCHUNK], f32, tag="wh")
                            nc.vector.tensor_mul(
                                w_t, exp_tiles[t].bitcast(f32)[:, hs], recip)
                            nc.sync.dma_start(w_d[mi * P:(mi + 1) * P, qsl], w_t)

    nc.finalize()
    return nc


def _get_program():
    if "nc" not in _CACHE:
        _CACHE["nc"] = _build_program()
    return _CACHE["nc"]


def kernel(x1, x2, Wq, bq, Wk, bk, Wv, bv):
    from concourse.bass_utils import run_bass_kernel_spmd

    nc = _get_program()
    in_maps = []
    for b in range(B):
        in_maps.append({
            "x1": np.ascontiguousarray(x1[b], dtype=np.float32),
            "x2": np.ascontiguousarray(x2[b], dtype=np.float32),
            "Wq": np.ascontiguousarray(Wq, dtype=np.float32),
            "bq": np.ascontiguousarray(bq, dtype=np.float32),
            "Wk": np.ascontiguousarray(Wk, dtype=np.float32),
            "bk": np.ascontiguousarray(bk, dtype=np.float32),
            "Wv": np.ascontiguousarray(Wv, dtype=np.float32),
            "bv": np.ascontiguousarray(bv, dtype=np.float32),
        })
    res = run_bass_kernel_spmd(nc, in_maps, core_ids=list(range(B)),
                               trace=_CACHE.get("trace", False))
    _CACHE["last_result"] = res
    out = np.stack([res.results[b]["out"] for b in range(B)])
    w = np.stack([res.results[b]["w"] for b in range(B)])
    return out, w


# revision 18
# speedup vs baseline: 1.0997x; 1.0588x over previous
"""Trainium2 Bass kernel for nn_Attention_41686952575337.

Computes, per batch b (B=8, one NeuronCore each, weights replicated):
    q = Wq @ x1[b] + bq            (K=128, Lq=2048)
    k = Wk @ x2[b] + bk            (K=128, Lk=2048)
    v = Wv @ x2[b] + bv            (O=128, Lk=2048)
    u = (k^T q) / sqrt(K)          (Lk, Lq)
    w = softmax(u, axis=0)         (softmax over Lk)
    out = v @ w                    (O, Lq)
returns (out, w) stacked over batch.

Strategy: scores live in (m=Lk on partitions, q on free) tiles. The
softmax-over-partitions sum runs on the TensorEngine with an all-ones
stationary operand (which also broadcasts the denominator across all 128
partitions); exp on ScalarE; normalize split between VectorE and GpSimd
(both are otherwise idle halves of the time). Big matmuls run as float32r
(full-rate fp32 path, ~1e-4 accuracy). m-tiles are processed in PAIRS
sharing one (128,1024) PSUM tile so ACT/DVE instructions are twice as
large (per-instruction overhead and semaphore traffic dominate otherwise).
Inputs are loaded in chunks with k projected first so the score stream
starts early; the last q-chunk is normalized at half-tile grain to shrink
the kernel tail.
"""

import numpy as np

B = 8
C = 128          # CQ = CM = K = O = 128
LQ = 2048
LK = 2048
P = 128
QCHUNK = 512     # q-chunk width (one PSUM bank)
NQ = LQ // QCHUNK
NM = LK // P     # 16 m-tiles
NPAIR = NM // 2  # m-tile pairs
SCALE = 1.0 / float(np.sqrt(np.float32(C)))

_CACHE = {}


def _build_program():
    import concourse.tile as tile
    from concourse import bacc, mybir
    from concourse.masks import make_identity

    f32 = mybir.dt.float32
    f32r = mybir.dt.float32r
    Exp = mybir.ActivationFunctionType.Exp
    Ident = mybir.ActivationFunctionType.Identity

    nc = bacc.Bacc(None, target_bir_lowering=False)

    x1 = nc.dram_tensor("x1", [C, LQ], f32, kind="ExternalInput")
    x2 = nc.dram_tensor("x2", [C, LK], f32, kind="ExternalInput")
    wq_d = nc.dram_tensor("Wq", [C, C], f32, kind="ExternalInput")
    bq_d = nc.dram_tensor("bq", [C], f32, kind="ExternalInput")
    wk_d = nc.dram_tensor("Wk", [C, C], f32, kind="ExternalInput")
    bk_d = nc.dram_tensor("bk", [C], f32, kind="ExternalInput")
    wv_d = nc.dram_tensor("Wv", [C, C], f32, kind="ExternalInput")
    bv_d = nc.dram_tensor("bv", [C], f32, kind="ExternalInput")
    out_d = nc.dram_tensor("out", [C, LQ], f32, kind="ExternalOutput")
    w_d = nc.dram_tensor("w", [LK, LQ], f32, kind="ExternalOutput")
    # view for paired stores: (m-pair, row, half, q)
    w_pairs = w_d.rearrange("(t h p) q -> t p h q", h=2, p=P)

    with tile.TileContext(nc) as tc:
        with (
            tc.tile_pool(name="singles", bufs=1) as singles,
            tc.tile_pool(name="big", bufs=1) as big,
            tc.tile_pool(name="exps", bufs=3) as exps,
            tc.tile_pool(name="post", bufs=2) as post,
            tc.tile_pool(name="wout", bufs=8) as wout,
            tc.tile_pool(name="pu", bufs=3, space="PSUM") as pu,
            tc.tile_pool(name="pacc", bufs=1, space="PSUM") as pacc,
        ):
            # ---- constants / weights ----
            ident = singles.tile([P, P], f32)
            make_identity(nc, ident)
            ones_f = singles.tile([P, P], f32)
            nc.vector.memset(ones_f, 1.0)
            ones = singles.tile([P, P], f32r)
            nc.vector.tensor_copy(ones, ones_f)

            x1_sb = big.tile([C, LQ], f32, tag="x1")
            x2_sb = big.tile([C, LK], f32, tag="x2")
            for j in range(NQ):
                sl = slice(j * QCHUNK, (j + 1) * QCHUNK)
                nc.sync.dma_start(x2_sb[:, sl], x2[:, sl])
            wgt = singles.tile([P, 3, C], f32)       # Wq | Wk | Wv
            nc.sync.dma_start(wgt[:, 0, :], wq_d[:])
            nc.sync.dma_start(wgt[:, 1, :], wk_d[:])
            nc.sync.dma_start(wgt[:, 2, :], wv_d[:])
            bias = singles.tile([P, 3], f32)         # bq | bk | bv
            nc.sync.dma_start(bias[:, 0:1], bq_d[:, None])
            nc.sync.dma_start(bias[:, 1:2], bk_d[:, None])
            nc.sync.dma_start(bias[:, 2:3], bv_d[:, None])
            bqs = singles.tile([P, 1], f32)          # bq / sqrt(K)
            nc.vector.tensor_scalar_mul(bqs, bias[:, 0:1], SCALE)

            # transpose weights (lhsT layout: contraction on partitions)
            wgtT = singles.tile([P, 3, C], f32)      # WqT | WkT | WvT
            ptw = pu.tile([P, 2 * QCHUNK], f32, tag="pu")
            for i in range(3):
                nc.tensor.transpose(ptw[:, i * P:(i + 1) * P], wgt[:, i, :], ident)
            nc.any.tensor_copy(wgtT[:], ptw[:, :3 * P].rearrange("p (i c) -> p i c", i=3))

            # ---- chunked input loads; x2/k first so the score stream can
            # start as early as possible ----
            for j in range(NQ):
                sl = slice(j * QCHUNK, (j + 1) * QCHUNK)
                nc.sync.dma_start(x1_sb[:, sl], x1[:, sl])

            # ---- projections (exact fp32 matmuls; ACT rounds to f32r) ----
            q_sb = big.tile([P, LQ], f32r, tag="q")
            k_sb = big.tile([P, LK], f32r, tag="k")
            v_sb = big.tile([P, LK], f32, tag="v")
            for (dst, wi, b_ap, scl, src) in (
                (k_sb, 1, bias[:, 1:2], 1.0, x2_sb),
                (v_sb, 2, bias[:, 2:3], 1.0, x2_sb),
                (q_sb, 0, bqs, SCALE, x1_sb),
            ):
                for j in range(NQ // 2):
                    pp = pu.tile([P, 2 * QCHUNK], f32, tag="pu")
                    for h in range(2):
                        sl = slice((2 * j + h) * QCHUNK, (2 * j + h + 1) * QCHUNK)
                        nc.tensor.matmul(pp[:, h * QCHUNK:(h + 1) * QCHUNK],
                                         wgtT[:, wi, :], src[:, sl],
                                         start=True, stop=True)
                    dsl = slice(2 * j * QCHUNK, 2 * (j + 1) * QCHUNK)
                    nc.scalar.activation(dst[:, dsl], pp, Ident, bias=b_ap, scale=scl)

            # ---- v^T tiles: vT[:, mi*128:+128] = v[:, mi-chunk]^T ----
            vT_sb = big.tile([P, LK], f32r, tag="vT")
            for g in range(2):
                ptv = pu.tile([P, 2 * QCHUNK], f32, tag="pu")
                for i in range(8):
                    mi = g * 8 + i
                    nc.tensor.transpose(ptv[:, i * P:(i + 1) * P],
                                        v_sb[:, mi * P:(mi + 1) * P], ident)
                nc.any.tensor_copy(vT_sb[:, g * 8 * P:(g + 1) * 8 * P], ptv)

            # ---- main loop over q-chunks ----
            for qc in range(NQ):
                qsl = slice(qc * QCHUNK, (qc + 1) * QCHUNK)
                po = pacc.tile([P, QCHUNK], f32, tag="po")    # out accum (unnorm)
                pd = pacc.tile([P, QCHUNK], f32, tag="pd")    # denominator (bcast)
                exp_tiles = []
                for t in range(NPAIR):
                    pu_t = pu.tile([P, 2 * QCHUNK], f32, tag="pu")
                    for h in range(2):
                        msl = slice((2 * t + h) * P, (2 * t + h + 1) * P)
                        nc.tensor.matmul(pu_t[:, h * QCHUNK:(h + 1) * QCHUNK],
                                         k_sb[:, msl], q_sb[:, qsl],
                                         start=True, stop=True)
                    e_t = exps.tile([P, 2 * QCHUNK], f32r, tag=f"exp_{t}")
                    nc.scalar.activation(e_t, pu_t, Exp)
                    exp_tiles.append(e_t)
                    for h in range(2):
                        hs = slice(h * QCHUNK, (h + 1) * QCHUNK)
                        msl = slice((2 * t + h) * P, (2 * t + h + 1) * P)
                        first = t == 0 and h == 0
                        last = t == NPAIR - 1 and h == 1
                        nc.tensor.matmul(pd, ones, e_t[:, hs],
                                         start=first, stop=last)
                        nc.tensor.matmul(po, vT_sb[:, msl], e_t[:, hs],
                                         start=first, stop=last)
                recip = post.tile([P, QCHUNK], f32, tag="recip")
                nc.vector.reciprocal_approx_fast(out=recip, in_=pd)
                # out-normalize first: frees the po/pd accumulators for the
                # next chunk before the long w-normalize stream
                o_sb = post.tile([P, QCHUNK], f32, tag="osb")
                nc.vector.tensor_mul(o_sb, po, recip)
                nc.sync.dma_start(out_d[:, qsl], o_sb)
                # normalize + store w (DVE; GpSimd shares SBUF ports so
                # offloading there halves both engines); finer grain on the
                # last chunk to shrink the kernel tail
                if qc < NQ - 1:
                    rbc = recip[:, None, :].to_broadcast([P, 2, QCHUNK])
                    for t in range(NPAIR):
                        w_t = wout.tile([P, 2, QCHUNK], f32, tag="w")
                        nc.vector.tensor_mul(
                            w_t,
                            exp_tiles[t].bitcast(f32).rearrange("p (h q) -> p h q", h=2),
                            rbc)
                        nc.sync.dma_start(w_pairs[t, :, :, qsl], w_t)
                else:
                    for t in range(NPAIR):
                        for h in range(2):
                            hs = slice(h * QCHUNK, (h + 1) * QCHUNK)
                            mi = 2 * t + h
                            w_t = wout.tile([P, QCHUNK], f32, tag="wh")
                            nc.vector.tensor_mul(
                                w_t, exp_tiles[t].bitcast(f32)[:, hs], recip)
                            nc.sync.dma_start(w_d[mi * P:(mi + 1) * P, qsl], w_t)

    nc.finalize()
    return nc


def _get_program():
    if "nc" not in _CACHE:
        _CACHE["nc"] = _build_program()
    return _CACHE["nc"]


def kernel(x1, x2, Wq, bq, Wk, bk, Wv, bv):
    from concourse.bass_utils import run_bass_kernel_spmd

    nc = _get_program()
    in_maps = []
    for b in range(B):
        in_maps.append({
            "x1": np.ascontiguousarray(x1[b], dtype=np.float32),
            "x2": np.ascontiguousarray(x2[b], dtype=np.float32),
            "Wq": np.ascontiguousarray(Wq, dtype=np.float32),
            "bq": np.ascontiguousarray(bq, dtype=np.float32),
            "Wk": np.ascontiguousarray(Wk, dtype=np.float32),
            "bk": np.ascontiguousarray(bk, dtype=np.float32),
            "Wv": np.ascontiguousarray(Wv, dtype=np.float32),
            "bv": np.ascontiguousarray(bv, dtype=np.float32),
        })
    res = run_bass_kernel_spmd(nc, in_maps, core_ids=list(range(B)),
                               trace=_CACHE.get("trace", False))
    _CACHE["last_result"] = res
    out = np.stack([res.results[b]["out"] for b in range(B)])
    w = np.stack([res.results[b]["w"] for b in range(B)])
    return out, w
